# revision 7
# baseline (speedup 1.0000x reference)
"""Menghao PointTransformer classifier on Trainium2 (8 NeuronCores).

Sharding: data-parallel over batch — 32 clouds -> 4 clouds per core,
same NEFF on all 8 cores (SPMD), per-core input slices, host gathers.

Division of labor inside kernel():
  * host (numpy): FPS + argsort-KNN index computation (sequential,
    data-dependent control flow) and input marshaling.
  * device (Bass/Tile NEFF x8 cores): all dense compute — conv1/2,
    grouped local ops l0/l1 (on-device indirect-DMA gathers of the
    neighbor rows), 4 offset-attention layers, fuse conv, global max
    pool, MLP head.
"""
import sys
import numpy as np

if '/opt/trn_rl_repo' not in sys.path:
    sys.path.insert(0, '/opt/trn_rl_repo')

P = 128
NB = 4           # clouds per core
N0 = 4096        # input points per cloud
S1 = 512         # stage-1 samples
S2 = 256         # stage-2 samples
K = 32           # neighbors

_CACHE = {}


# ----------------------------------------------------------------------------
# host-side numpy replica (indices only)
# ----------------------------------------------------------------------------

def _bn(x, g, b):
    return x * (g / np.sqrt(np.float32(1.0 + 1e-5))) + b


def _fps(xyz, npoint):
    B, N, _ = xyz.shape
    dist = np.full((B, N), 1e10, dtype=xyz.dtype)
    farthest = np.zeros((B,), dtype=np.int64)
    idxs = np.zeros((B, npoint), dtype=np.int64)
    ar = np.arange(B)
    for i in range(npoint):
        idxs[:, i] = farthest
        centroid = xyz[ar, farthest]
        d = ((xyz - centroid[:, None, :]) ** 2).sum(-1)
        dist = np.minimum(dist, d)
        farthest = dist.argmax(-1)
    return idxs


def _cos(a, b):
    num = np.einsum('bsc,bnc->bsn', a.astype(np.float32), b.astype(np.float32))
    na = np.linalg.norm(a, axis=-1)
    nb = np.linalg.norm(b, axis=-1)
    return num / (na[:, :, None] * nb[:, None, :])


def _host_indices(x, p):
    xyz = np.ascontiguousarray(x[..., :3], dtype=np.float32)
    h = np.maximum(_bn(x @ p['conv1_w'].T, p['bn1_g'], p['bn1_b']), 0)
    h = np.maximum(_bn(h @ p['conv2_w'].T, p['bn2_g'], p['bn2_b']), 0)
    h = h.astype(np.float32)

    fps1 = _fps(xyz, S1)
    ar = np.arange(x.shape[0])[:, None]
    nx1 = xyz[ar, fps1]
    np1 = h[ar, fps1]
    d1 = 0.75 * _cos(nx1, xyz) + 0.25 * _cos(np1[..., 3:], h[..., 3:])
    idx1 = np.argsort(d1, axis=-1, kind='stable')[:, :, :K]

    grouped = h[ar[:, :, None], idx1]
    feat = np.concatenate([grouped - np1[:, :, None, :],
                           np.broadcast_to(np1[:, :, None, :], grouped.shape)],
                          axis=-1)
    f = np.maximum(_bn(feat @ p['l0_c1_w'].T, p['l0_bn1_g'], p['l0_bn1_b']), 0)
    f = np.maximum(_bn(f @ p['l0_c2_w'].T, p['l0_bn2_g'], p['l0_bn2_b']), 0)
    f0 = f.max(axis=2).astype(np.float32)

    fps2 = _fps(nx1, S2)
    nx2 = nx1[ar, fps2]
    np2 = f0[ar, fps2]
    d2 = 0.75 * _cos(nx2, nx1) + 0.25 * _cos(np2[..., 3:], f0[..., 3:])
    idx2 = np.argsort(d2, axis=-1, kind='stable')[:, :, :K]
    return fps1, idx1, fps2, idx2


# ----------------------------------------------------------------------------
# device kernel
# ----------------------------------------------------------------------------

def _build_nc():
    import concourse.bass as bass
    import concourse.mybir as mybir
    from concourse.tile import TileContext

    FP32 = mybir.dt.float32
    F32R = mybir.dt.float32r
    I32 = mybir.dt.int32
    AX = mybir.AxisListType
    OP = mybir.AluOpType
    AF = mybir.ActivationFunctionType

    nc = bass.Bass()

    xT = nc.dram_tensor("xT", [3, NB * N0], FP32, kind="ExternalInput")
    idx1 = nc.dram_tensor("idx1", [P, 16 * K], I32, kind="ExternalInput")
    cidx1 = nc.dram_tensor("cidx1", [P, 16], I32, kind="ExternalInput")
    idx2 = nc.dram_tensor("idx2", [P, 8 * K], I32, kind="ExternalInput")
    cidx2 = nc.dram_tensor("cidx2", [P, 8], I32, kind="ExternalInput")

    def win(name, shape):
        return nc.dram_tensor(name, shape, FP32, kind="ExternalInput")

    # weights, pre-transposed/blocked on host (see _prep_weights)
    w1T = win("w1T", [3, 64]); b1s = win("b1s", [64, 2])
    w2T = win("w2T", [64, 64]); b2s = win("b2s", [64, 2])
    l0aT = win("l0aT", [64, 128]); l0dT = win("l0dT", [64, 128]); l0b1 = win("l0b1", [128, 2])
    l0c2T = win("l0c2T", [128, 128]); l0b2 = win("l0b2", [128, 2])
    l1aT = win("l1aT", [128, 256]); l1dT = win("l1dT", [128, 256]); l1b1 = win("l1b1", [128, 2 * 2])
    l1c2T = win("l1c2T", [128, 2 * 256]); l1b2 = win("l1b2", [128, 2 * 2])
    sac1T = win("sac1T", [128, 2 * 256]); sab1 = win("sab1", [128, 2 * 2])
    sac2T = win("sac2T", [128, 2 * 256]); sab2 = win("sab2", [128, 2 * 2])
    saqkT = [win(f"sa{i}qkT", [128, 2 * 64]) for i in range(4)]
    savT = [win(f"sa{i}vT", [128, 2 * 256]) for i in range(4)]
    savb = [win(f"sa{i}vb", [1, 256]) for i in range(4)]
    satT = [win(f"sa{i}tT", [128, 2 * 256]) for i in range(4)]
    sabn = [win(f"sa{i}bn", [128, 2 * 2]) for i in range(4)]
    fuseT = win("fuseT", [128, 10 * 1024]); fuseb = win("fuseb", [128, 8 * 2])
    lin1T = win("lin1T", [128, 8 * 512]); lin1b = win("lin1b", [128, 4 * 2])
    lin2T = win("lin2T", [128, 4 * 256]); lin2b = win("lin2b", [128, 2 * 2])
    lin3T = win("lin3T", [128, 2 * 40]); lin3b = win("lin3b", [40, 1])
    ident_in = win("ident_in", [P, P])

    out = nc.dram_tensor("out", [40, NB], FP32, kind="ExternalOutput")

    hT_d = nc.dram_tensor("hT_d", [64, NB * N0], FP32, kind="Internal")
    hP_d = nc.dram_tensor("hP_d", [NB * N0, 64], FP32, kind="Internal")
    f0P_d = nc.dram_tensor("f0P_d", [NB * S1, 128], FP32, kind="Internal")
    f1T_d = nc.dram_tensor("f1T_d", [128, 2 * NB * S2], FP32, kind="Internal")

    def bc_mid(ap, n, at):
        """insert a step-0 dim of size n at position `at` of ap's free dims"""
        aps = [list(d) for d in ap.ap]
        aps.insert(at, [0, n])
        return bass.AP(ap.tensor, ap.offset, aps)

    with TileContext(nc) as tc:
        with tc.tile_pool(name="wpool", bufs=1) as wp, \
             tc.tile_pool(name="sbuf", bufs=3) as sb, \
             tc.tile_pool(name="big", bufs=1) as bigp, \
             tc.tile_pool(name="psA", bufs=2, space="PSUM") as psA, \
             tc.tile_pool(name="psB", bufs=2, space="PSUM") as psB, \
             tc.tile_pool(name="psC", bufs=2, space="PSUM") as psC:

            def wtile(dram, name, dtype=FP32):
                t = wp.tile(dram.shape, dtype, tag=name)
                nc.sync.dma_start(t[:], dram[:])
                return t

            W1 = wtile(w1T, "W1"); B1 = wtile(b1s, "B1")
            W2 = wtile(w2T, "W2"); B2 = wtile(b2s, "B2")
            L0A = wtile(l0aT, "L0A"); L0D = wtile(l0dT, "L0D"); L0B1 = wtile(l0b1, "L0B1")
            L0C2 = wtile(l0c2T, "L0C2"); L0B2 = wtile(l0b2, "L0B2")
            L1A = wtile(l1aT, "L1A"); L1D = wtile(l1dT, "L1D"); L1B1 = wtile(l1b1, "L1B1")
            L1C2 = wtile(l1c2T, "L1C2"); L1B2 = wtile(l1b2, "L1B2")
            SAC1 = wtile(sac1T, "SAC1"); SAB1 = wtile(sab1, "SAB1")
            SAC2 = wtile(sac2T, "SAC2"); SAB2 = wtile(sab2, "SAB2")
            QKW = [wtile(saqkT[i], f"QKW{i}") for i in range(4)]
            VW = [wtile(savT[i], f"VW{i}") for i in range(4)]
            VBR = [wtile(savb[i], f"VBR{i}") for i in range(4)]
            TW = [wtile(satT[i], f"TW{i}") for i in range(4)]
            BNW = [wtile(sabn[i], f"BNW{i}") for i in range(4)]
            FUSEB = wtile(fuseb, "FUSEB")
            LIN1B = wtile(lin1b, "LIN1B")
            LIN2 = wtile(lin2T, "LIN2"); LIN2B = wtile(lin2b, "LIN2B")
            LIN3 = wtile(lin3T, "LIN3"); LIN3B = wtile(lin3b, "LIN3B")
            IDENT = wtile(ident_in, "IDENT")
            IDX1 = wtile(idx1, "IDX1", I32)
            CIDX1 = wtile(cidx1, "CIDX1", I32)
            IDX2 = wtile(idx2, "IDX2", I32)
            CIDX2 = wtile(cidx2, "CIDX2", I32)
            ONES = wp.tile([P, 1], FP32, tag="ONES")
            nc.vector.memset(ONES[:], 1.0)
            ONESROW = wp.tile([1, P], FP32, tag="ONESROW")
            nc.vector.memset(ONESROW[:], 1.0)

            def r(x):
                return x  # fp32 matmuls (f32r needs rounded producers)

            def evict(dst, src, scale_ap, bias_ap, relu=True):
                nc.scalar.activation(dst, src,
                                     AF.Relu if relu else AF.Identity,
                                     scale=scale_ap, bias=bias_ap)

            # ================= S1: conv1+conv2 =================
            NCH = NB * N0
            for j in range(0, NCH, 512):
                xt = sb.tile([3, 512], FP32, tag="xt")
                nc.sync.dma_start(xt[:], xT[:, j:j + 512])
                p1 = psA.tile([64, 512], FP32, tag="mmA")
                nc.tensor.matmul(p1[:], r(W1[:]), r(xt[:]), start=True, stop=True)
                h1 = sb.tile([64, 512], FP32, tag="h1")
                evict(h1[:], p1[:], B1[:, 0:1], B1[:, 1:2])
                p2 = psB.tile([64, 512], FP32, tag="mmB")
                nc.tensor.matmul(p2[:], r(W2[:]), r(h1[:]), start=True, stop=True)
                h2 = sb.tile([64, 512], FP32, tag="h2")
                evict(h2[:], p2[:], B2[:, 0:1], B2[:, 1:2])
                nc.sync.dma_start(hT_d[:, j:j + 512], h2[:])
                # transposed copy to hP (point-major) via PE
                hpt = sb.tile([P, 4, 64], FP32, tag="hpt")
                for kk in range(4):
                    tp = psC.tile([P, 64], FP32, tag="tp")
                    nc.tensor.transpose(tp[:], h2[:, kk * P:(kk + 1) * P], IDENT[:64, :64])
                    nc.scalar.activation(hpt[:, kk, :], tp[:], AF.Copy)
                nc.sync.dma_start(
                    hP_d[j:j + 512, :].rearrange("(kk p) c -> p kk c", p=P), hpt[:])

            # ================= S2: l0 =================
            for t in range(16):
                G = bigp.tile([P, K, 64], FP32, tag="G")
                for kk in range(K):
                    nc.gpsimd.indirect_dma_start(
                        out=G[:, kk, :], out_offset=None, in_=hP_d[:],
                        in_offset=bass.IndirectOffsetOnAxis(
                            ap=IDX1[:, t * K + kk:t * K + kk + 1], axis=0))
                CTR = sb.tile([P, 64], FP32, tag="CTR")
                nc.gpsimd.indirect_dma_start(
                    out=CTR[:], out_offset=None, in_=hP_d[:],
                    in_offset=bass.IndirectOffsetOnAxis(ap=CIDX1[:, t:t + 1], axis=0))
                tpc = psC.tile([64, P], FP32, tag="tp")
                nc.tensor.transpose(tpc[:], CTR[:], IDENT[:])
                CTRf = sb.tile([64, P], FP32, tag="CTRf")
                nc.scalar.activation(CTRf[:], tpc[:], AF.Copy)
                ctr_rep = bc_mid(CTRf[:], 4, 1)  # [64, 4, 128] step-0
                F0t = sb.tile([128, P], FP32, tag="F0t")
                for jc in range(0, K * P, 512):
                    Gfc = sb.tile([64, 512], FP32, tag="Gfc")
                    for kx in range(4):
                        kk = jc // P + kx
                        tp = psC.tile([64, P], FP32, tag="tp")
                        nc.tensor.transpose(tp[:], G[:, kk, :], IDENT[:])
                        nc.scalar.activation(Gfc[:, kx * P:(kx + 1) * P], tp[:], AF.Copy)
                    pc = psA.tile([128, 512], FP32, tag="mmA")
                    nc.tensor.matmul(pc[:], r(L0A[:]), r(Gfc[:]),
                                     start=True, stop=False)
                    nc.tensor.matmul(pc[:], r(L0D[:]), ctr_rep, start=False, stop=True)
                    H1c = sb.tile([128, 512], FP32, tag="H1c")
                    evict(H1c[:], pc[:], L0B1[:, 0:1], L0B1[:, 1:2])
                    pc2 = psB.tile([128, 512], FP32, tag="mmB")
                    nc.tensor.matmul(pc2[:], r(L0C2[:]), r(H1c[:]),
                                     start=True, stop=True)
                    h2t = sb.tile([128, 512], FP32, tag="h2t")
                    evict(h2t[:], pc2[:], L0B2[:, 0:1], L0B2[:, 1:2])
                    mx = sb.tile([128, P], FP32, tag="mx")
                    nc.vector.tensor_reduce(
                        mx[:], h2t[:].rearrange("c (k p) -> c p k", k=4),
                        axis=AX.X, op=OP.max)
                    if jc == 0:
                        nc.vector.tensor_copy(F0t[:], mx[:])
                    else:
                        nc.vector.tensor_tensor(F0t[:], F0t[:], mx[:], op=OP.max)
                # write point-major f0P rows (queries of this tile)
                tpf = psC.tile([P, 128], FP32, tag="tp")
                nc.tensor.transpose(tpf[:], F0t[:], IDENT[:])
                F0p = sb.tile([P, 128], FP32, tag="F0p")
                nc.scalar.activation(F0p[:], tpf[:], AF.Copy)
                nc.sync.dma_start(f0P_d[t * P:(t + 1) * P, :], F0p[:])

            # ================= S3: l1 =================
            for t in range(8):
                G2 = bigp.tile([P, K, 128], FP32, tag="G")
                for kk in range(K):
                    nc.gpsimd.indirect_dma_start(
                        out=G2[:, kk, :], out_offset=None, in_=f0P_d[:],
                        in_offset=bass.IndirectOffsetOnAxis(
                            ap=IDX2[:, t * K + kk:t * K + kk + 1], axis=0))
                CTR2 = sb.tile([P, 128], FP32, tag="CTR2")
                nc.gpsimd.indirect_dma_start(
                    out=CTR2[:], out_offset=None, in_=f0P_d[:],
                    in_offset=bass.IndirectOffsetOnAxis(ap=CIDX2[:, t:t + 1], axis=0))
                tpc2 = psC.tile([128, P], FP32, tag="tp")
                nc.tensor.transpose(tpc2[:], CTR2[:], IDENT[:])
                CTR2f = sb.tile([128, P], FP32, tag="CTR2f")
                nc.scalar.activation(CTR2f[:], tpc2[:], AF.Copy)
                ctr2_rep = bc_mid(CTR2f[:], 4, 1)
                F1t = sb.tile([128, 2, P], FP32, tag="F1t")
                for jc in range(0, K * P, 512):
                    G2fc = sb.tile([128, 512], FP32, tag="Gfc")
                    for kx in range(4):
                        kk = jc // P + kx
                        tp2 = psC.tile([128, P], FP32, tag="tp")
                        nc.tensor.transpose(tp2[:], G2[:, kk, :], IDENT[:])
                        nc.scalar.activation(G2fc[:, kx * P:(kx + 1) * P], tp2[:], AF.Copy)
                    H1c2 = sb.tile([128, 2, 512], FP32, tag="H1c2")
                    for mh in range(2):
                        pb = psA.tile([128, 512], FP32, tag="mmA")
                        nc.tensor.matmul(pb[:],
                                         r(L1A[:, mh * 128:(mh + 1) * 128]),
                                         r(G2fc[:]), start=True, stop=False)
                        nc.tensor.matmul(pb[:],
                                         r(L1D[:, mh * 128:(mh + 1) * 128]),
                                         ctr2_rep, start=False, stop=True)
                        evict(H1c2[:, mh, :], pb[:],
                              L1B1[:, 2 * mh:2 * mh + 1], L1B1[:, 2 * mh + 1:2 * mh + 2])
                    for mh in range(2):
                        pb2 = psB.tile([128, 512], FP32, tag="mmB")
                        for kh in range(2):
                            nc.tensor.matmul(
                                pb2[:],
                                r(L1C2[:, kh * 256 + mh * 128:kh * 256 + (mh + 1) * 128]),
                                r(H1c2[:, kh, :]),
                                start=(kh == 0), stop=(kh == 1))
                        h2b = sb.tile([128, 512], FP32, tag="h2t")
                        evict(h2b[:], pb2[:],
                              L1B2[:, 2 * mh:2 * mh + 1], L1B2[:, 2 * mh + 1:2 * mh + 2])
                        mx2 = sb.tile([128, P], FP32, tag="mx")
                        nc.vector.tensor_reduce(
                            mx2[:], h2b[:].rearrange("c (k p) -> c p k", k=4),
                            axis=AX.X, op=OP.max)
                        if jc == 0:
                            nc.vector.tensor_copy(F1t[:, mh, :], mx2[:])
                        else:
                            nc.vector.tensor_tensor(F1t[:, mh, :], F1t[:, mh, :],
                                                    mx2[:], op=OP.max)
                for mh in range(2):
                    nc.sync.dma_start(
                        f1T_d[:, mh * NB * S2 + t * P: mh * NB * S2 + (t + 1) * P],
                        F1t[:, mh, :])

            # ================= S4: attention + fuse per cloud =================
            GOUT = wp.tile([128, 8, NB], FP32, tag="GOUT")
            for b in range(NB):
                F1 = bigp.tile([128, 2, S2], FP32, tag="F1")
                for mh in range(2):
                    # f1T_d cols within half mh: queries t*128+p ordered cloud-major
                    nc.sync.dma_start(F1[:, mh, :],
                                      f1T_d[:, mh * NB * S2 + b * S2:
                                            mh * NB * S2 + (b + 1) * S2])
                XA = bigp.tile([128, 2, S2], FP32, tag="XA")
                for mh in range(2):
                    pa = psA.tile([128, S2], FP32, tag="mmA")
                    for kh in range(2):
                        nc.tensor.matmul(
                            pa[:],
                            r(SAC1[:, kh * 256 + mh * 128:kh * 256 + (mh + 1) * 128]),
                            r(F1[:, kh, :]), start=(kh == 0), stop=(kh == 1))
                    evict(XA[:, mh, :], pa[:],
                          SAB1[:, 2 * mh:2 * mh + 1], SAB1[:, 2 * mh + 1:2 * mh + 2])
                XC = bigp.tile([128, 2, S2], FP32, tag="XC")
                for mh in range(2):
                    pa2 = psA.tile([128, S2], FP32, tag="mmA")
                    for kh in range(2):
                        nc.tensor.matmul(
                            pa2[:],
                            r(SAC2[:, kh * 256 + mh * 128:kh * 256 + (mh + 1) * 128]),
                            r(XA[:, kh, :]), start=(kh == 0), stop=(kh == 1))
                    evict(XC[:, mh, :], pa2[:],
                          SAB2[:, 2 * mh:2 * mh + 1], SAB2[:, 2 * mh + 1:2 * mh + 2])
                xcur = XC
                xlayers = []
                for li in range(4):
                    qk = sb.tile([64, S2], FP32, tag="qk")
                    pq = psB.tile([64, S2], FP32, tag="mmB")
                    for kh in range(2):
                        nc.tensor.matmul(pq[:], r(QKW[li][:, kh * 64:(kh + 1) * 64]),
                                         r(xcur[:, kh, :]), start=(kh == 0), stop=(kh == 1))
                    nc.scalar.activation(qk[:], pq[:], AF.Copy)
                    ATT = bigp.tile([128, 2, S2], FP32, tag="ATT")
                    SS = psC.tile([1, S2], FP32, tag="ss")
                    for ih in range(2):
                        pe = psB.tile([128, S2], FP32, tag="mmB")
                        nc.tensor.matmul(pe[:], r(qk[:, ih * 128:(ih + 1) * 128]),
                                         r(qk[:]), start=True, stop=True)
                        em = sb.tile([128, 1], FP32, tag="em")
                        nc.vector.tensor_reduce(em[:], pe[:], axis=AX.X, op=OP.max)
                        nsc = sb.tile([128, 1], FP32, tag="nsc")
                        nc.vector.tensor_scalar_mul(nsc[:], em[:], -1.0)
                        ex = sb.tile([128, S2], FP32, tag="ex")
                        es = sb.tile([128, 1], FP32, tag="es")
                        nc.scalar.activation(ex[:], pe[:], AF.Exp,
                                             bias=nsc[:, 0:1], accum_out=es[:])
                        rs = sb.tile([128, 1], FP32, tag="rs")
                        nc.vector.reciprocal(rs[:], es[:])
                        nc.vector.tensor_scalar_mul(ATT[:, ih, :], ex[:], rs[:, 0:1])
                        nc.tensor.matmul(SS[:], r(ONES[:]), r(ATT[:, ih, :]),
                                         start=(ih == 0), stop=(ih == 1))
                    SSr = sb.tile([1, S2], FP32, tag="SSr")
                    nc.scalar.activation(SSr[:], SS[:], AF.Copy)
                    SSe = sb.tile([1, S2], FP32, tag="SSe")
                    nc.vector.tensor_scalar_add(SSe[:], SSr[:], 1e-9)
                    SR = sb.tile([1, S2], FP32, tag="SR")
                    nc.vector.reciprocal(SR[:], SSe[:])
                    SREP = psC.tile([128, S2], FP32, tag="ss")
                    nc.tensor.matmul(SREP[:], r(ONESROW[:]), r(SR[:]), start=True, stop=True)
                    SREPs = sb.tile([128, S2], FP32, tag="SREPs")
                    nc.scalar.activation(SREPs[:], SREP[:], AF.Copy)
                    VP = bigp.tile([128, 2, 256], FP32, tag="VP")
                    for ih in range(2):
                        pv = psB.tile([128, 256], FP32, tag="mmB")
                        for kh in range(2):
                            nc.tensor.matmul(
                                pv[:], r(xcur[:, kh, ih * 128:(ih + 1) * 128]),
                                r(VW[li][:, kh * 256:(kh + 1) * 256]),
                                start=(kh == 0), stop=(kh == 1))
                        nc.scalar.activation(VP[:, ih, :], pv[:], AF.Copy)
                    XN = bigp.tile([128, 2, S2], FP32, tag=f"xn{li}")
                    for mh in range(2):
                        px = psA.tile([128, S2], FP32, tag="mmA")
                        for ih in range(2):
                            nc.tensor.matmul(px[:],
                                             r(VP[:, ih, mh * 128:(mh + 1) * 128]),
                                             r(ATT[:, ih, :]), start=(ih == 0), stop=False)
                        nc.tensor.matmul(px[:],
                                         r(VBR[li][:, mh * 128:(mh + 1) * 128]),
                                         r(SSr[:]), start=False, stop=True)
                        xm = sb.tile([128, S2], FP32, tag="xm")
                        nc.vector.tensor_tensor(xm[:], px[:], SREPs[:], op=OP.mult)
                        nc.vector.tensor_tensor(xm[:], xcur[:, mh, :], xm[:], op=OP.subtract)
                        nc.vector.tensor_copy(XN[:, mh, :], xm[:])
                    # trans proj + bn relu + residual (in place on XN)
                    XO = bigp.tile([128, 2, S2], FP32, tag=f"xo{li}")
                    for mh in range(2):
                        pt_ = psA.tile([128, S2], FP32, tag="mmA")
                        for kh in range(2):
                            nc.tensor.matmul(
                                pt_[:],
                                r(TW[li][:, kh * 256 + mh * 128:kh * 256 + (mh + 1) * 128]),
                                r(XN[:, kh, :]), start=(kh == 0), stop=(kh == 1))
                        tb = sb.tile([128, S2], FP32, tag="tb")
                        evict(tb[:], pt_[:],
                              BNW[li][:, 2 * mh:2 * mh + 1], BNW[li][:, 2 * mh + 1:2 * mh + 2])
                        nc.vector.tensor_tensor(XO[:, mh, :], xcur[:, mh, :], tb[:], op=OP.add)
                    xlayers.append(XO)
                    xcur = XO
                # fuse
                cat_blocks = [xlayers[0][:, 0, :], xlayers[0][:, 1, :],
                              xlayers[1][:, 0, :], xlayers[1][:, 1, :],
                              xlayers[2][:, 0, :], xlayers[2][:, 1, :],
                              xlayers[3][:, 0, :], xlayers[3][:, 1, :],
                              F1[:, 0, :], F1[:, 1, :]]
                for mt in range(8):
                    FUSEmt = sb.tile([128, 10, 128], FP32, tag="FUSEmt")
                    nc.sync.dma_start(
                        FUSEmt[:],
                        fuseT[:].rearrange("p (kc n) -> p kc n", kc=10)[:, :, mt * 128:(mt + 1) * 128])
                    pf = psA.tile([128, S2], FP32, tag="mmA")
                    for kc in range(10):
                        nc.tensor.matmul(
                            pf[:], r(FUSEmt[:, kc, :]),
                            r(cat_blocks[kc]), start=(kc == 0), stop=(kc == 9))
                    fb = sb.tile([128, S2], FP32, tag="fb")
                    evict(fb[:], pf[:],
                          FUSEB[:, 2 * mt:2 * mt + 1], FUSEB[:, 2 * mt + 1:2 * mt + 2],
                          relu=False)
                    lk = sb.tile([128, S2], FP32, tag="lk")
                    nc.vector.scalar_tensor_tensor(lk[:], fb[:], 0.2, fb[:],
                                                   op0=OP.mult, op1=OP.max)
                    nc.vector.tensor_reduce(GOUT[:, mt, b:b + 1], lk[:],
                                            axis=AX.X, op=OP.max)

            # ================= S5: head =================
            g1 = wp.tile([128, 4, NB], FP32, tag="g1")
            for mt in range(4):
                LIN1mt = sb.tile([128, 8, 128], FP32, tag="LIN1mt")
                nc.sync.dma_start(
                    LIN1mt[:],
                    lin1T[:].rearrange("p (kc n) -> p kc n", kc=8)[:, :, mt * 128:(mt + 1) * 128])
                ph = psA.tile([128, NB], FP32, tag="mmA")
                for kc in range(8):
                    nc.tensor.matmul(
                        ph[:], r(LIN1mt[:, kc, :]),
                        r(GOUT[:, kc, :]), start=(kc == 0), stop=(kc == 7))
                evict(g1[:, mt, :], ph[:],
                      LIN1B[:, 2 * mt:2 * mt + 1], LIN1B[:, 2 * mt + 1:2 * mt + 2])
            g2 = wp.tile([128, 2, NB], FP32, tag="g2")
            for mt in range(2):
                ph2 = psA.tile([128, NB], FP32, tag="mmA")
                for kc in range(4):
                    nc.tensor.matmul(
                        ph2[:], r(LIN2[:, kc * 256 + mt * 128:kc * 256 + (mt + 1) * 128]),
                        r(g1[:, kc, :]), start=(kc == 0), stop=(kc == 3))
                evict(g2[:, mt, :], ph2[:],
                      LIN2B[:, 2 * mt:2 * mt + 1], LIN2B[:, 2 * mt + 1:2 * mt + 2])
            ph3 = psA.tile([40, NB], FP32, tag="mmA")
            for kc in range(2):
                nc.tensor.matmul(ph3[:], r(LIN3[:, kc * 40:(kc + 1) * 40]),
                                 r(g2[:, kc, :]), start=(kc == 0), stop=(kc == 1))
            outt = sb.tile([40, NB], FP32, tag="outt")
            b3 = LIN3B[:]
            b3b = bass.AP(b3.tensor, b3.offset, [list(b3.ap[0]), [0, NB]])
            nc.vector.tensor_tensor(outt[:], ph3[:], b3b, op=OP.add)
            nc.sync.dma_start(out[:], outt[:])

    return nc


def _prep_weights(p):
    """host-side weight marshaling into the device layouts (fp32)."""
    E = np.float32(np.sqrt(np.float32(1.0 + 1e-5)))

    def sc(g):
        return (g / E).astype(np.float32)

    def blockT(w):
        # w [O, I] -> lhsT blocks layout [128, (I//128)*O] : WT[I,O] row-blocked
        wt = np.ascontiguousarray(w.T, dtype=np.float32)  # [I, O]
        I, O = wt.shape
        assert I % 128 == 0
        return np.ascontiguousarray(
            wt.reshape(I // 128, 128, O).transpose(1, 0, 2).reshape(128, -1))

    def bnpack(g, b, nblk):
        s = sc(g); bb = b.astype(np.float32)
        ch = s.shape[0]
        if nblk == 0:   # channels <= 128, [ch, 2]
            return np.stack([s, bb], axis=1).astype(np.float32)
        # [128, nblk*2] : per block (scale, bias) column pairs
        s2 = s.reshape(nblk, 128).T
        b2 = bb.reshape(nblk, 128).T
        outp = np.zeros((128, nblk * 2), np.float32)
        for m in range(nblk):
            outp[:, 2 * m] = s2[:, m]
            outp[:, 2 * m + 1] = b2[:, m]
        return outp

    d = {}
    d['w1T'] = np.ascontiguousarray(p['conv1_w'].T.astype(np.float32))
    d['b1s'] = bnpack(p['bn1_g'], p['bn1_b'], 0)
    d['w2T'] = np.ascontiguousarray(p['conv2_w'].T.astype(np.float32))
    d['b2s'] = bnpack(p['bn2_g'], p['bn2_b'], 0)
    w = p['l0_c1_w'].astype(np.float32)                # [128, 128in]
    d['l0aT'] = np.ascontiguousarray(w[:, :64].T)      # [64, 128]
    d['l0dT'] = np.ascontiguousarray((w[:, 64:] - w[:, :64]).T)
    d['l0b1'] = bnpack(p['l0_bn1_g'], p['l0_bn1_b'], 0)
    d['l0c2T'] = np.ascontiguousarray(p['l0_c2_w'].T.astype(np.float32))
    d['l0b2'] = bnpack(p['l0_bn2_g'], p['l0_bn2_b'], 0)
    w = p['l1_c1_w'].astype(np.float32)                # [256, 256in]
    d['l1aT'] = np.ascontiguousarray(w[:, :128].T)     # [128, 256]
    d['l1dT'] = np.ascontiguousarray((w[:, 128:] - w[:, :128]).T)
    d['l1b1'] = bnpack(p['l1_bn1_g'], p['l1_bn1_b'], 2)
    d['l1c2T'] = blockT(p['l1_c2_w'])
    d['l1b2'] = bnpack(p['l1_bn2_g'], p['l1_bn2_b'], 2)
    d['sac1T'] = blockT(p['sa_c1_w'])
    d['sab1'] = bnpack(p['sa_bn1_g'], p['sa_bn1_b'], 2)
    d['sac2T'] = blockT(p['sa_c2_w'])
    d['sab2'] = bnpack(p['sa_bn2_g'], p['sa_bn2_b'], 2)
    for i in range(1, 5):
        pre = 'sa%d' % i
        d[f'sa{i-1}qkT'] = blockT(p[pre + '_qk_w'])
        d[f'sa{i-1}vT'] = blockT(p[pre + '_v_w'])
        d[f'sa{i-1}vb'] = p[pre + '_v_b'].astype(np.float32).reshape(1, 256)
        d[f'sa{i-1}tT'] = blockT(p[pre + '_t_w'])
        s = sc(p[pre + '_bn_g'])
        bias = p[pre + '_t_b'].astype(np.float32) * s + p[pre + '_bn_b'].astype(np.float32)
        d[f'sa{i-1}bn'] = bnpack(p[pre + '_bn_g'],
                                 bias * E / p[pre + '_bn_g'] if False else None, 2) \
            if False else None
        # direct pack: scale=s, bias=bias
        s2 = s.reshape(2, 128).T
        b2 = bias.reshape(2, 128).T
        o = np.zeros((128, 4), np.float32)
        o[:, 0] = s2[:, 0]; o[:, 1] = b2[:, 0]; o[:, 2] = s2[:, 1]; o[:, 3] = b2[:, 1]
        d[f'sa{i-1}bn'] = o
    d['fuseT'] = blockT(p['fuse_w'])
    d['fuseb'] = bnpack(p['fuse_bn_g'], p['fuse_bn_b'], 8)
    d['lin1T'] = blockT(p['lin1_w'])
    d['lin1b'] = bnpack(p['bn6_g'], p['bn6_b'], 4)
    d['lin2T'] = blockT(p['lin2_w'])
    s7 = sc(p['bn7_g'])
    bias7 = p['lin2_b'].astype(np.float32) * s7 + p['bn7_b'].astype(np.float32)
    s2_ = s7.reshape(2, 128).T; b2_ = bias7.reshape(2, 128).T
    o = np.zeros((128, 4), np.float32)
    o[:, 0] = s2_[:, 0]; o[:, 1] = b2_[:, 0]; o[:, 2] = s2_[:, 1]; o[:, 3] = b2_[:, 1]
    d['lin2b'] = o
    d['lin3T'] = blockT(p['lin3_w'])
    d['lin3b'] = p['lin3_b'].astype(np.float32).reshape(40, 1)
    d['ident_in'] = np.eye(P, dtype=np.float32)
    return d


def _fix_excess_waits(nc, max_waits=1):
    import concourse.mybir as mybir
    n_fix = 0
    for f in nc.m.functions:
        for bb in f.blocks:
            newlist = []
            for inst in bb.instructions:
                si = inst.sync_info
                if si and si.on_wait and len(si.on_wait) > max_waits:
                    waits = list(si.on_wait)
                    k = 0
                    while len(waits) - k > max_waits:
                        chunk = waits[k:k + max_waits]
                        k += max_waits
                        nop = mybir.InstNoOp(name=f'I-waitfix-{n_fix}')
                        n_fix += 1
                        nop.engine = inst.engine
                        nop.sync_info = mybir.SyncInfo(on_wait=chunk, on_update=[])
                        newlist.append(nop)
                    si.on_wait = waits[k:]
                    inst.sync_info = si
                newlist.append(inst)
            bb.instructions = newlist
    return n_fix


def kernel(x, params):
    x = np.asarray(x, dtype=np.float32)
    p = {k: np.asarray(v, dtype=np.float32) for k, v in params.items()}
    B = x.shape[0]
    assert B == 32 and x.shape[1] == N0

    fps1, idx1, fps2, idx2 = _host_indices(x, p)
    wd = _prep_weights(p)

    if 'nc' not in _CACHE:
        nc = _build_nc()
        _fix_excess_waits(nc)
        _CACHE['nc'] = nc
    nc = _CACHE['nc']

    xyz = np.ascontiguousarray(x[..., :3])
    in_maps = []
    for core in range(8):
        bs = range(core * NB, (core + 1) * NB)
        # xT [3, NB*4096] cloud-major cols
        xTc = np.concatenate([xyz[b].T for b in bs], axis=1)
        xTc = np.ascontiguousarray(xTc, dtype=np.float32)
        # idx layouts: M-tile t=(b_local, qchunk); query p -> m = t*128+p
        i1 = np.zeros((P, 16 * K), np.int32)
        c1 = np.zeros((P, 16), np.int32)
        for bl in range(NB):
            for qc in range(4):
                t = bl * 4 + qc
                rows = idx1[bs[bl], qc * 128:(qc + 1) * 128]       # [128, K]
                i1[:, t * K:(t + 1) * K] = rows + bl * N0
                c1[:, t] = fps1[bs[bl], qc * 128:(qc + 1) * 128] + bl * N0
        i2 = np.zeros((P, 8 * K), np.int32)
        c2 = np.zeros((P, 8), np.int32)
        for bl in range(NB):
            for qc in range(2):
                t = bl * 2 + qc
                rows = idx2[bs[bl], qc * 128:(qc + 1) * 128]
                i2[:, t * K:(t + 1) * K] = rows + bl * S1
                c2[:, t] = fps2[bs[bl], qc * 128:(qc + 1) * 128] + bl * S1
        m = {'xT': xTc, 'idx1': i1, 'cidx1': c1, 'idx2': i2, 'cidx2': c2}
        m.update(wd)
        in_maps.append(m)

    from concourse.bass_utils import run_bass_kernel_spmd
    res = run_bass_kernel_spmd(nc, in_maps, core_ids=list(range(8)))

    out = np.zeros((32, 40), np.float32)
    for core in range(8):
        oc = res.results[core]['out']          # [40, NB]
        for bl in range(NB):
            out[core * NB + bl] = oc[:, bl]
    return out
